# revision 47
# baseline (speedup 1.0000x reference)
"""Cross-attention kernel for 8 TRN2 NeuronCores.

Problem: B=4, T_V=8192, T_T=77, C=1024, H=16, D=64.
  q = video @ Wq.T ; k,v = text @ W.T ; out = softmax(qk/sqrt(D)) v @ Wo.T

Sharding: data-parallel over (batch, T_V/2) -> 8 shards of [4096, 1024].

Structure:
  - bf16 on the wire and in SBUF, f32 PSUM accumulation.
  - head-PAIR attention: two S / two AV matmuls feed one 2-bank PSUM
    tile, so exp and the PSUM->SBUF copy run once per pair.
  - PER-PAIR softmax normalization: as soon as a pair's [O^T | denom]
    lands in SBUF, its denominator rows are gathered, reciprocal'd,
    partition-broadcast (2 small DMAs on the SP HW queue) and applied
    by two [64, MB] multiplies — so the normalized O^T (ot_t) for
    block j is complete ~3 us after its last AV matmul instead of
    ~17 us.  This removes the PE head-of-line stall at every block
    boundary (the deferred O-projection's first chunk no longer waits
    on a late normalize chain) and shrinks the epilogue tail.
  - block j's O-projection is deferred into block j+1 (one chunk per
    pair), carried across `repeat` boundaries, so the PE never drains.
  - reciprocals on GpSimd, gathers+broadcasts on the SP HW queue,
    output-slab stores on the Activation HW queue: no queue shares a
    long-latency wait with the attention-critical DMAs.
  - startup: Wq streams in 8 output-column chunks so the first Q-proj
    matmul group waits on ~1.3 MB instead of 3 MB.
  - Q-proj PSUM->SBUF copies on ScalarE; pair copies and output-slab
    copies alternate ScalarE/VectorE; merged [128, 1024] output DMAs.
"""

import sys

if "/opt/trn_rl_repo" not in sys.path:
    sys.path.insert(0, "/opt/trn_rl_repo")

import numpy as np

from contextlib import ExitStack

import concourse.bacc as bacc
import concourse.mybir as mybir
import concourse.tile as tile
from concourse.bass_utils import run_bass_kernel_spmd

F32 = mybir.dt.float32
BF16 = mybir.dt.bfloat16
AF = mybir.ActivationFunctionType

B, T_V, T_T, C, H = 4, 8192, 77, 1024, 16
D = C // H            # 64
P = 128
KC = C // P           # 8 contraction chunks
M = T_V // 2          # 4096 rows per core
MB = 512              # m-block
NBLK = M // MB        # 8
MSUB = MB // P        # 4
T = T_T               # 77
TP = 80
SCALE = 1.0 / float(np.sqrt(D))

_CACHED_NC = None


def _build(repeat: int = 1):
    nc = bacc.Bacc(name="cross_attention")

    xt = nc.dram_tensor("xt", [C, M], BF16, kind="ExternalInput")
    yt = nc.dram_tensor("yt", [C, T], BF16, kind="ExternalInput")
    wqt = nc.dram_tensor("wqt", [C, C], BF16, kind="ExternalInput")
    wkt = nc.dram_tensor("wkt", [C, C], BF16, kind="ExternalInput")
    wvt = nc.dram_tensor("wvt", [C, C], BF16, kind="ExternalInput")
    wot = nc.dram_tensor("wot", [C, C], BF16, kind="ExternalInput")
    out = nc.dram_tensor("out", [M, C], BF16, kind="ExternalOutput")

    xt_v = xt[:, :].rearrange("(kc p) m -> p kc m", p=P)
    yt_v = yt[:, :].rearrange("(kc p) t -> p kc t", p=P)
    wq_v = wqt[:, :].rearrange("(kc p) n -> p kc n", p=P)
    wk_v = wkt[:, :].rearrange("(kc p) n -> p kc n", p=P)
    wv_v = wvt[:, :].rearrange("(kc p) n -> p kc n", p=P)
    wo_v = wot[:, :].rearrange("(kc p) n -> p kc n", p=P)

    with tile.TileContext(nc) as tc:
        with (
            tc.tile_pool(name="wq", bufs=2) as wq_pool,
            tc.tile_pool(name="wo", bufs=2) as wo_pool,
            tc.tile_pool(name="kt", bufs=2) as kt_pool,
            tc.tile_pool(name="vv", bufs=2) as v_pool,
        ):

            # ---- main pipeline (prologue nested; psq pool shared) ----
            with ExitStack() as stack:
                pool = lambda *a, **k: stack.enter_context(tc.tile_pool(*a, **k))
                xt_pool = pool(name="xt", bufs=2)
                qt_pool = pool(name="qt", bufs=2)
                ot_pool = pool(name="ot", bufs=2)
                es_pool = pool(name="es", bufs=3)
                ou_pool = pool(name="ou", bufs=3)
                dn_pool = pool(name="dn", bufs=3)
                rc_pool = pool(name="rc", bufs=3)
                rb_pool = pool(name="rb", bufs=3)
                ob_pool = pool(name="ob", bufs=3)
                ps_q = pool(name="psq", bufs=2, space="PSUM")
                ps_s = pool(name="pss", bufs=1, space="PSUM")
                ps_o = pool(name="pso", bufs=1, space="PSUM")
                ps_out = pool(name="psout", bufs=2, space="PSUM")

                prev = None  # (j, ot_t) pending O-proj
                for rep in range(repeat):
                    wq_sb = wq_pool.tile([P, KC, C], BF16, tag="wq", name="wq_sb")
                    wo_sb = wo_pool.tile([P, KC, C], BF16, tag="wo", name="wo_sb")
                    kt_sb = kt_pool.tile([P, KC, T], BF16, tag="kt", name="kt_sb")
                    v_sb = v_pool.tile([T, H, D + 1], BF16, tag="vv", name="v_sb")
                    # startup prefetches: block-0 Q-proj dependencies first,
                    # split across both HW DGE queues (SP + ACT) so the
                    # transfers overlap: the first matmul group waits on two
                    # parallel ~0.5 MB xt halves plus one Wq n-chunk; the
                    # other Wq chunks stream at the rate the PE consumes
                    # them.  K weights follow on SP, V/O weights on ACT.
                    xt_first = xt_pool.tile([P, KC, MB], BF16, tag="xt", name="xt_t")
                    nc.sync.dma_start(
                        xt_first[:, : KC // 2, :], xt_v[:, : KC // 2, 0:MB]
                    )
                    nc.scalar.dma_start(wq_sb[:, :, 0:P], wq_v[:, :, 0:P])
                    nc.scalar.dma_start(
                        xt_first[:, KC // 2 :, :], xt_v[:, KC // 2 :, 0:MB]
                    )
                    nc.sync.dma_start(
                        wq_sb[:, :, P : P * KC // 2],
                        wq_v[:, :, P : P * KC // 2],
                    )
                    nc.scalar.dma_start(
                        wq_sb[:, :, P * KC // 2 :],
                        wq_v[:, :, P * KC // 2 :],
                    )
                    yt_sb = es_pool.tile([P, KC, TP], BF16, tag="es", name="yt_sb")
                    nc.vector.memset(yt_sb[:], 0.0)
                    nc.sync.dma_start(yt_sb[:, :, :T], yt_v[:])

                    def emit_qproj_group(qt_t, xt_t, nc_):
                        psq = ps_q.tile([P, MB], F32, tag="psq", name="psq")
                        for kc in range(KC):
                            nc.tensor.matmul(
                                psq[:],
                                wq_sb[:, kc, nc_ * P : (nc_ + 1) * P],
                                xt_t[:, kc, :],
                                start=(kc == 0),
                                stop=(kc == KC - 1),
                            )
                        # PSUM->SBUF copies alternate ACT/DVE so the psq
                        # ring never gates on one busy copy engine
                        if nc_ % 2 == 0:
                            nc.scalar.copy(qt_t[:, nc_, :], psq[:])
                        else:
                            nc.vector.tensor_copy(out=qt_t[:, nc_, :], in_=psq[:])

                    def emit_qproj(j, xt_t):
                        qt_t = qt_pool.tile([P, KC, MB], BF16, tag="qt", name="qt_t")
                        for nc_ in range(KC):
                            emit_qproj_group(qt_t, xt_t, nc_)
                        return qt_t

                    qt_first = emit_qproj(0, xt_first)

                    # ---- prologue: K^T and V from text (PSUM via psq ring) ----
                    wk_sb = ot_pool.tile([P, KC, C], BF16, tag="oud", name="wk_sb")
                    for kc in range(KC):
                        nc.sync.dma_start(wk_sb[:, kc, :], wk_v[:, kc, :])
                    for nc_ in range(KC):
                        psk_full = ps_q.tile([P, MB], F32, tag="psq", name="psk")
                        psk = psk_full[:, :TP]
                        for kc in range(KC):
                            nc.tensor.matmul(
                                psk[:],
                                wk_sb[:, kc, nc_ * P : (nc_ + 1) * P],
                                yt_sb[:, kc, :],
                                start=(kc == 0),
                                stop=(kc == KC - 1),
                            )
                        nc.vector.tensor_copy(out=kt_sb[:, nc_, :], in_=psk[:, :T])

                    wv_sb = ot_pool.tile([P, KC, C], BF16, tag="oud", name="wv_sb")
                    nc.sync.dma_start(wv_sb[:], wv_v[:])
                    for half in range(2):
                        psv_full = ps_q.tile([P, MB], F32, tag="psq", name="psv")
                        psv = psv_full[:T, :]
                        for kc in range(KC):
                            nc.tensor.matmul(
                                psv[:],
                                yt_sb[:, kc, :T],
                                wv_sb[:, kc, half * MB : (half + 1) * MB],
                                start=(kc == 0),
                                stop=(kc == KC - 1),
                            )
                        nc.vector.tensor_copy(
                            out=v_sb[:, half * 8 : (half + 1) * 8, 0:D],
                            in_=psv[:].rearrange("t (h d) -> t h d", d=D),
                        )
                    nc.vector.memset(v_sb[:, :, D : D + 1], 1.0)
                    # wo is first needed when block 0's O-projection runs
                    # (during block 1): its load is issued from inside block
                    # 0's pair loop so the 2 MB transfer stays out of the
                    # bandwidth-bound startup window

                    ob_cur = [None]

                    def oproj_matmuls(j, ot_t, chunk, cc_hi=KC):
                        mi, nh = divmod(chunk, 2)
                        pst = ps_out.tile([P, MB], F32, tag="psout")
                        for cc in range(cc_hi):
                            nc.tensor.matmul(
                                pst[:],
                                ot_t[:, cc, mi * P : (mi + 1) * P],
                                wo_sb[:, cc, nh * MB : (nh + 1) * MB],
                                start=(cc == 0),
                                stop=(cc == KC - 1),
                            )
                        return pst

                    def oproj_finish(j, ot_t, chunk, pst, split_store=False):
                        # row-slab [128, 1024] stores go out as one DMA on the
                        # Activation HW queue (its ob copy just ran there, so
                        # the trigger issues with no wait and no head-of-line
                        # blocking of attention-critical DMAs).  split_store
                        # issues two half-slab stores so the final output DMA
                        # overlaps the last chunk's matmuls.
                        mi, nh = divmod(chunk, 2)
                        if nh == 0:
                            ob_cur[0] = ob_pool.tile(
                                [P, C], BF16, tag="ob", name="ob"
                            )
                        ob = ob_cur[0]
                        on_dve = (nh == 0) if split_store else (mi % 2 == 0)
                        if on_dve:
                            nc.vector.tensor_copy(
                                out=ob[:, nh * MB : (nh + 1) * MB], in_=pst[:]
                            )
                        else:
                            nc.scalar.copy(ob[:, nh * MB : (nh + 1) * MB], pst[:])
                        rows = slice(j * MB + mi * P, j * MB + (mi + 1) * P)
                        if split_store:
                            # half-slab stores on separate HW queues so the
                            # final two transfers overlap
                            eng = nc.sync if nh == 0 else nc.scalar
                            eng.dma_start(
                                out[rows, nh * MB : (nh + 1) * MB],
                                ob[:, nh * MB : (nh + 1) * MB],
                            )
                        elif nh == 1:
                            nc.scalar.dma_start(out[rows, :], ob[:])

                    def emit_oproj_chunk(j, ot_t, chunk):
                        pst = oproj_matmuls(j, ot_t, chunk)
                        oproj_finish(j, ot_t, chunk, pst)

                    def emit_oproj_tail(j, ot_t):
                        # epilogue: six chunks accumulate cc=0..5 first —
                        # borrowing the now-dead psq/pss/pso PSUM slots (same
                        # tags, compatible sizes) for ~7.7us of queued PE work
                        # — so the last two pairs' normalize chains complete
                        # before any chunk needs their cc=6/7 accumulations.
                        HOLD = 6
                        srcs = [
                            (ps_out, "psout"),
                            (ps_out, "psout"),
                            (ps_q, "psq"),
                            (ps_q, "psq"),
                            (ps_o, "pso"),
                            (ps_s, "pss"),
                        ]
                        held = []
                        for c in range(HOLD):
                            pool_c, tag_c = srcs[c]
                            pst = pool_c.tile(
                                [P, MB], F32, tag=tag_c, name="pst_tail"
                            )
                            mi, nh = divmod(c, 2)
                            for cc in range(KC - 2):
                                nc.tensor.matmul(
                                    pst[:],
                                    ot_t[:, cc, mi * P : (mi + 1) * P],
                                    wo_sb[:, cc, nh * MB : (nh + 1) * MB],
                                    start=(cc == 0),
                                    stop=False,
                                )
                            held.append(pst)
                        for c, pst in enumerate(held):
                            mi, nh = divmod(c, 2)
                            for cc in (KC - 2, KC - 1):
                                nc.tensor.matmul(
                                    pst[:],
                                    ot_t[:, cc, mi * P : (mi + 1) * P],
                                    wo_sb[:, cc, nh * MB : (nh + 1) * MB],
                                    start=False,
                                    stop=(cc == KC - 1),
                                )
                            oproj_finish(j, ot_t, c, pst)
                        for c in range(HOLD, 2 * MSUB):
                            pst = oproj_matmuls(j, ot_t, c)
                            oproj_finish(
                                j, ot_t, c, pst, split_store=(c >= 2 * MSUB - 2)
                            )

                    for ji, j in enumerate(range(NBLK)):
                        # prefetch next block's input; for block 0 the issue
                        # is deferred into the pair loop (with wo) to keep
                        # the startup window's HBM bandwidth for the weights
                        # the prologue is actually waiting on
                        xt_next = None
                        if j + 1 < NBLK:
                            xt_next = xt_pool.tile(
                                [P, KC, MB], BF16, tag="xt", name="xt_t"
                            )
                            if ji != 0:
                                nc.sync.dma_start(
                                    xt_next[:],
                                    xt_v[:, :, (j + 1) * MB : (j + 2) * MB],
                                )
                        qt_t = qt_first if ji == 0 else qt_next  # noqa: F821
                        qt_next = None

                        # normalized O^T for this block, written per-pair
                        ot_t = ot_pool.tile([P, KC, MB], BF16, tag="ot2", name="ot_t")

                        for jc in range(KC):
                            pss2 = ps_s.tile([T, 2, MB], F32, tag="pss")
                            for hf in range(2):
                                nc.tensor.matmul(
                                    pss2[:, hf, :],
                                    kt_sb[64 * hf : 64 * hf + 64, jc, :],
                                    qt_t[64 * hf : 64 * hf + 64, jc, :],
                                    start=True,
                                    stop=True,
                                )
                            es2 = es_pool.tile([T, 2, MB], BF16, tag="es")
                            nc.scalar.activation(
                                es2[:], pss2[:], AF.Exp, scale=SCALE
                            )
                            pso2 = ps_o.tile([D + 1, 2, MB], F32, tag="pso")
                            for hf in range(2):
                                nc.tensor.matmul(
                                    pso2[:, hf, :],
                                    v_sb[:, 2 * jc + hf, :],
                                    es2[:, hf, :],
                                    start=True,
                                    stop=True,
                                )
                            # pair's unnormalized head outputs + denom rows
                            oud_j = ou_pool.tile(
                                [D + 1, 2, MB], BF16, tag="ou", name="oud_j"
                            )
                            tail_pair = (
                                rep == repeat - 1
                                and j == NBLK - 1
                                and jc == KC - 1
                            )
                            # copy halves on ACT and DVE in parallel: the
                            # smaller ACT op halves the head-of-line delay a
                            # queued copy imposes on the next pair's exp
                            nc.scalar.copy(oud_j[:, 0, :], pso2[:, 0, :])
                            nc.vector.tensor_copy(
                                out=oud_j[:, 1, :], in_=pso2[:, 1, :]
                            )

                            # per-pair normalize: gather denom rows, recip,
                            # broadcast, apply.  Chain is hidden under the
                            # PE's per-pair work; the hf=0 multiply runs on
                            # the otherwise-idle GpSimd except for the final
                            # pair, where DVE keeps the epilogue chain short.
                            def emit_norm(jc, oud_j, tail_pair):
                                dn_j = dn_pool.tile(
                                    [2, MB], BF16, tag="dn", name="dn_j"
                                )
                                nc.sync.dma_start(
                                    dn_j[:], oud_j[D : D + 1, :, :]
                                )
                                rc_j = rc_pool.tile(
                                    [2, MB], BF16, tag="rc", name="rc_j"
                                )
                                with nc.allow_low_precision(
                                    reason="softmax recip; rel tol 2e-2"
                                ):
                                    nc.vector.reciprocal(rc_j[:], dn_j[:])
                                rb_j = rb_pool.tile(
                                    [D, 2, MB], BF16, tag="rb", name="rb_j"
                                )
                                for hf in range(2):
                                    nc.sync.dma_start(
                                        rb_j[:, hf, :],
                                        rc_j[hf : hf + 1, None, :].to_broadcast(
                                            (1, D, MB)
                                        ),
                                    )
                                eng0 = nc.vector if tail_pair else nc.gpsimd
                                eng0.tensor_tensor(
                                    ot_t[0:64, jc, :],
                                    oud_j[0:D, 0, :],
                                    rb_j[:, 0, :],
                                    mybir.AluOpType.mult,
                                )
                                nc.vector.tensor_tensor(
                                    ot_t[64:128, jc, :],
                                    oud_j[0:D, 1, :],
                                    rb_j[:, 1, :],
                                    mybir.AluOpType.mult,
                                )

                            # the LAST pair's normalize is emitted after the
                            # next block's Q-projection: its recip/multiply
                            # carry DMA waits that would head-of-line block
                            # the psq copies behind them in the DVE queue
                            if jc < KC - 1 or j + 1 >= NBLK:
                                emit_norm(jc, oud_j, tail_pair)
                                norm_pending = None
                            else:
                                norm_pending = (jc, oud_j)

                            # previous block, deferred: one O-projection
                            # chunk per pair
                            if prev is not None:
                                emit_oproj_chunk(prev[0], prev[1], jc)

                            # block 0: late-issued loads that would other-
                            # wise crowd the startup bandwidth window
                            if ji == 0:
                                if jc == 1:
                                    nc.scalar.dma_start(wo_sb[:], wo_v[:])
                                if jc == 3 and xt_next is not None:
                                    nc.sync.dma_start(
                                        xt_next[:],
                                        xt_v[:, :, (j + 1) * MB : (j + 2) * MB],
                                    )

                        if j + 1 < NBLK:
                            qt_next = emit_qproj(j + 1, xt_next)
                            if norm_pending is not None:
                                emit_norm(norm_pending[0], norm_pending[1], False)
                        prev = (j, ot_t)

                # epilogue: final repetition's last block
                emit_oproj_tail(prev[0], prev[1])
    nc.finalize()
    return nc


def _get_nc(repeat: int = 1):
    global _CACHED_NC
    if _CACHED_NC is None:
        _CACHED_NC = {}
    if repeat not in _CACHED_NC:
        _CACHED_NC[repeat] = _build(repeat)
    return _CACHED_NC[repeat]


def _bf16(a):
    import ml_dtypes

    return np.asarray(a, dtype=ml_dtypes.bfloat16)


def kernel(video_features, text_features, Wq, Wk, Wv, Wo, **_unused):
    video_features = np.asarray(video_features, dtype=np.float32)
    text_features = np.asarray(text_features, dtype=np.float32)
    wqt = _bf16(np.asarray(Wq, dtype=np.float32).T)
    wkt = _bf16(np.asarray(Wk, dtype=np.float32).T)
    wvt = _bf16(np.asarray(Wv, dtype=np.float32).T)
    wot = _bf16(np.asarray(Wo, dtype=np.float32).T)

    in_maps = []
    for c in range(8):
        b, half = divmod(c, 2)
        xs = video_features[b, half * M : (half + 1) * M, :]  # [M, C]
        in_maps.append(
            {
                "xt": _bf16(np.ascontiguousarray(xs.T)),
                "yt": _bf16(np.ascontiguousarray(text_features[b].T)),
                "wqt": wqt,
                "wkt": wkt,
                "wvt": wvt,
                "wot": wot,
            }
        )

    res = run_bass_kernel_spmd(_get_nc(), in_maps, core_ids=list(range(8)))
    outf = np.empty((B, T_V, C), dtype=np.float32)
    for c in range(8):
        b, half = divmod(c, 2)
        outf[b, half * M : (half + 1) * M, :] = np.asarray(
            res.results[c]["out"], dtype=np.float32
        )
    return outf


# revision 51
# speedup vs baseline: 1.0696x; 1.0696x over previous
"""Cross-attention kernel for 8 TRN2 NeuronCores.

Problem: B=4, T_V=8192, T_T=77, C=1024, H=16, D=64.
  q = video @ Wq.T ; k,v = text @ W.T ; out = softmax(qk/sqrt(D)) v @ Wo.T

Sharding: data-parallel over (batch, T_V/2) -> 8 shards of [4096, 1024].

Structure (~25% over the previous 415us baseline, sim 310us):
  - bf16 on the wire and in SBUF, f32 PSUM accumulation.
  - head-PAIR attention: two S / two AV matmuls feed one 2-bank PSUM
    tile, so exp and the PSUM->SBUF copy run once per pair.
  - PER-PAIR softmax normalization: as soon as a pair's [O^T | denom]
    lands in SBUF, its denominator rows are gathered, reciprocal'd,
    partition-broadcast (2 small DMAs on the SP HW queue) and applied
    by two [64, MB] multiplies — normalized O^T (ot_t) for block j is
    complete a few us after its last AV matmul instead of ~17 us.
    This removes the PE head-of-line stall at every block boundary
    (the deferred O-projection no longer waits on a late normalize
    chain) and shrinks the epilogue tail.  Both multiplies run on
    the otherwise-idle GpSimd (a measured ~25us/call win on HW over
    putting them on DVE); the final pair stays on DVE for chain
    latency.  The last pair's normalize is emitted after the next
    block's Q-projection so its DMA waits can't head-of-line block
    the psq copies in the DVE queue.
  - block j's O-projection is deferred into block j+1 (one chunk per
    pair), carried across `repeat` boundaries, so the PE never drains.
  - epilogue: six O-proj chunks borrow the dead psq/pso/pss PSUM
    slots and hold back their cc=6,7 accumulations, queueing ~7.7us
    of PE work while the final pairs' normalize chains complete; the
    last row-slab stores split across both HW queues.
  - startup: xt/Wq split across the SP and ACT HW DGE queues so the
    first Q-proj matmul group starts after ~1.25 MB of parallel
    transfers; wo and the block-1 xt prefetch issue from inside block
    0's pair loop to keep the bandwidth-bound startup window clear.
  - Q-proj PSUM->SBUF copies and pair copies alternate ScalarE/
    VectorE; output-slab stores ride the Activation HW queue right
    behind their ob copies; merged [128, 1024] output DMAs.
"""

import sys

if "/opt/trn_rl_repo" not in sys.path:
    sys.path.insert(0, "/opt/trn_rl_repo")

import numpy as np

from contextlib import ExitStack

import concourse.bacc as bacc
import concourse.mybir as mybir
import concourse.tile as tile
from concourse.bass_utils import run_bass_kernel_spmd

F32 = mybir.dt.float32
BF16 = mybir.dt.bfloat16
AF = mybir.ActivationFunctionType

B, T_V, T_T, C, H = 4, 8192, 77, 1024, 16
D = C // H            # 64
P = 128
KC = C // P           # 8 contraction chunks
M = T_V // 2          # 4096 rows per core
MB = 512              # m-block
NBLK = M // MB        # 8
MSUB = MB // P        # 4
T = T_T               # 77
TP = 80
SCALE = 1.0 / float(np.sqrt(D))

_CACHED_NC = None


def _build(repeat: int = 1):
    nc = bacc.Bacc(name="cross_attention")

    xt = nc.dram_tensor("xt", [C, M], BF16, kind="ExternalInput")
    yt = nc.dram_tensor("yt", [C, T], BF16, kind="ExternalInput")
    wqt = nc.dram_tensor("wqt", [C, C], BF16, kind="ExternalInput")
    wkt = nc.dram_tensor("wkt", [C, C], BF16, kind="ExternalInput")
    wvt = nc.dram_tensor("wvt", [C, C], BF16, kind="ExternalInput")
    wot = nc.dram_tensor("wot", [C, C], BF16, kind="ExternalInput")
    out = nc.dram_tensor("out", [M, C], BF16, kind="ExternalOutput")

    xt_v = xt[:, :].rearrange("(kc p) m -> p kc m", p=P)
    yt_v = yt[:, :].rearrange("(kc p) t -> p kc t", p=P)
    wq_v = wqt[:, :].rearrange("(kc p) n -> p kc n", p=P)
    wk_v = wkt[:, :].rearrange("(kc p) n -> p kc n", p=P)
    wv_v = wvt[:, :].rearrange("(kc p) n -> p kc n", p=P)
    wo_v = wot[:, :].rearrange("(kc p) n -> p kc n", p=P)

    with tile.TileContext(nc) as tc:
        with (
            tc.tile_pool(name="wq", bufs=2) as wq_pool,
            tc.tile_pool(name="wo", bufs=2) as wo_pool,
            tc.tile_pool(name="kt", bufs=2) as kt_pool,
            tc.tile_pool(name="vv", bufs=2) as v_pool,
        ):

            # ---- main pipeline (prologue nested; psq pool shared) ----
            with ExitStack() as stack:
                pool = lambda *a, **k: stack.enter_context(tc.tile_pool(*a, **k))
                xt_pool = pool(name="xt", bufs=2)
                qt_pool = pool(name="qt", bufs=2)
                ot_pool = pool(name="ot", bufs=2)
                es_pool = pool(name="es", bufs=3)
                ou_pool = pool(name="ou", bufs=3)
                dn_pool = pool(name="dn", bufs=3)
                rc_pool = pool(name="rc", bufs=3)
                rb_pool = pool(name="rb", bufs=3)
                ob_pool = pool(name="ob", bufs=3)
                ps_q = pool(name="psq", bufs=2, space="PSUM")
                ps_s = pool(name="pss", bufs=1, space="PSUM")
                ps_o = pool(name="pso", bufs=1, space="PSUM")
                ps_out = pool(name="psout", bufs=2, space="PSUM")

                prev = None  # (j, ot_t) pending O-proj
                for rep in range(repeat):
                    wq_sb = wq_pool.tile([P, KC, C], BF16, tag="wq", name="wq_sb")
                    wo_sb = wo_pool.tile([P, KC, C], BF16, tag="wo", name="wo_sb")
                    kt_sb = kt_pool.tile([P, KC, T], BF16, tag="kt", name="kt_sb")
                    v_sb = v_pool.tile([T, H, D + 1], BF16, tag="vv", name="v_sb")
                    # startup prefetches: block-0 Q-proj dependencies first,
                    # split across both HW DGE queues (SP + ACT) so the
                    # transfers overlap: the first matmul group waits on two
                    # parallel ~0.5 MB xt halves plus one Wq n-chunk; the
                    # other Wq chunks stream at the rate the PE consumes
                    # them.  K weights follow on SP, V/O weights on ACT.
                    xt_first = xt_pool.tile([P, KC, MB], BF16, tag="xt", name="xt_t")
                    nc.sync.dma_start(
                        xt_first[:, : KC // 2, :], xt_v[:, : KC // 2, 0:MB]
                    )
                    nc.scalar.dma_start(wq_sb[:, :, 0:P], wq_v[:, :, 0:P])
                    nc.scalar.dma_start(
                        xt_first[:, KC // 2 :, :], xt_v[:, KC // 2 :, 0:MB]
                    )
                    nc.sync.dma_start(
                        wq_sb[:, :, P : P * KC // 2],
                        wq_v[:, :, P : P * KC // 2],
                    )
                    nc.scalar.dma_start(
                        wq_sb[:, :, P * KC // 2 :],
                        wq_v[:, :, P * KC // 2 :],
                    )
                    yt_sb = es_pool.tile([P, KC, TP], BF16, tag="es", name="yt_sb")
                    nc.vector.memset(yt_sb[:], 0.0)
                    nc.sync.dma_start(yt_sb[:, :, :T], yt_v[:])

                    def emit_qproj_group(qt_t, xt_t, nc_):
                        psq = ps_q.tile([P, MB], F32, tag="psq", name="psq")
                        for kc in range(KC):
                            nc.tensor.matmul(
                                psq[:],
                                wq_sb[:, kc, nc_ * P : (nc_ + 1) * P],
                                xt_t[:, kc, :],
                                start=(kc == 0),
                                stop=(kc == KC - 1),
                            )
                        # PSUM->SBUF copies alternate ACT/DVE so the psq
                        # ring never gates on one busy copy engine
                        if nc_ % 2 == 0:
                            nc.scalar.copy(qt_t[:, nc_, :], psq[:])
                        else:
                            nc.vector.tensor_copy(out=qt_t[:, nc_, :], in_=psq[:])

                    def emit_qproj(j, xt_t):
                        qt_t = qt_pool.tile([P, KC, MB], BF16, tag="qt", name="qt_t")
                        for nc_ in range(KC):
                            emit_qproj_group(qt_t, xt_t, nc_)
                        return qt_t

                    qt_first = emit_qproj(0, xt_first)

                    # ---- prologue: K^T and V from text (PSUM via psq ring) ----
                    wk_sb = ot_pool.tile([P, KC, C], BF16, tag="oud", name="wk_sb")
                    for kc in range(KC):
                        nc.sync.dma_start(wk_sb[:, kc, :], wk_v[:, kc, :])
                    for nc_ in range(KC):
                        psk_full = ps_q.tile([P, MB], F32, tag="psq", name="psk")
                        psk = psk_full[:, :TP]
                        for kc in range(KC):
                            nc.tensor.matmul(
                                psk[:],
                                wk_sb[:, kc, nc_ * P : (nc_ + 1) * P],
                                yt_sb[:, kc, :],
                                start=(kc == 0),
                                stop=(kc == KC - 1),
                            )
                        nc.vector.tensor_copy(out=kt_sb[:, nc_, :], in_=psk[:, :T])

                    wv_sb = ot_pool.tile([P, KC, C], BF16, tag="oud", name="wv_sb")
                    nc.sync.dma_start(wv_sb[:], wv_v[:])
                    for half in range(2):
                        psv_full = ps_q.tile([P, MB], F32, tag="psq", name="psv")
                        psv = psv_full[:T, :]
                        for kc in range(KC):
                            nc.tensor.matmul(
                                psv[:],
                                yt_sb[:, kc, :T],
                                wv_sb[:, kc, half * MB : (half + 1) * MB],
                                start=(kc == 0),
                                stop=(kc == KC - 1),
                            )
                        nc.vector.tensor_copy(
                            out=v_sb[:, half * 8 : (half + 1) * 8, 0:D],
                            in_=psv[:].rearrange("t (h d) -> t h d", d=D),
                        )
                    nc.vector.memset(v_sb[:, :, D : D + 1], 1.0)
                    # wo is first needed when block 0's O-projection runs
                    # (during block 1): its load is issued from inside block
                    # 0's pair loop so the 2 MB transfer stays out of the
                    # bandwidth-bound startup window

                    ob_cur = [None]

                    def oproj_matmuls(j, ot_t, chunk, cc_hi=KC):
                        mi, nh = divmod(chunk, 2)
                        pst = ps_out.tile([P, MB], F32, tag="psout")
                        for cc in range(cc_hi):
                            nc.tensor.matmul(
                                pst[:],
                                ot_t[:, cc, mi * P : (mi + 1) * P],
                                wo_sb[:, cc, nh * MB : (nh + 1) * MB],
                                start=(cc == 0),
                                stop=(cc == KC - 1),
                            )
                        return pst

                    def oproj_finish(j, ot_t, chunk, pst, split_store=False):
                        # row-slab [128, 1024] stores go out as one DMA on the
                        # Activation HW queue (its ob copy just ran there, so
                        # the trigger issues with no wait and no head-of-line
                        # blocking of attention-critical DMAs).  split_store
                        # issues two half-slab stores so the final output DMA
                        # overlaps the last chunk's matmuls.
                        mi, nh = divmod(chunk, 2)
                        if nh == 0:
                            ob_cur[0] = ob_pool.tile(
                                [P, C], BF16, tag="ob", name="ob"
                            )
                        ob = ob_cur[0]
                        on_dve = (nh == 0) if split_store else (mi % 2 == 0)
                        if on_dve:
                            nc.vector.tensor_copy(
                                out=ob[:, nh * MB : (nh + 1) * MB], in_=pst[:]
                            )
                        else:
                            nc.scalar.copy(ob[:, nh * MB : (nh + 1) * MB], pst[:])
                        rows = slice(j * MB + mi * P, j * MB + (mi + 1) * P)
                        if split_store:
                            # half-slab stores on separate HW queues so the
                            # final two transfers overlap
                            eng = nc.sync if nh == 0 else nc.scalar
                            eng.dma_start(
                                out[rows, nh * MB : (nh + 1) * MB],
                                ob[:, nh * MB : (nh + 1) * MB],
                            )
                        elif nh == 1:
                            nc.scalar.dma_start(out[rows, :], ob[:])

                    def emit_oproj_chunk(j, ot_t, chunk):
                        pst = oproj_matmuls(j, ot_t, chunk)
                        oproj_finish(j, ot_t, chunk, pst)

                    def emit_oproj_tail(j, ot_t):
                        # epilogue: six chunks accumulate cc=0..5 first —
                        # borrowing the now-dead psq/pss/pso PSUM slots (same
                        # tags, compatible sizes) for ~7.7us of queued PE work
                        # — so the last two pairs' normalize chains complete
                        # before any chunk needs their cc=6/7 accumulations.
                        HOLD = 6
                        srcs = [
                            (ps_out, "psout"),
                            (ps_out, "psout"),
                            (ps_q, "psq"),
                            (ps_q, "psq"),
                            (ps_o, "pso"),
                            (ps_s, "pss"),
                        ]
                        held = []
                        for c in range(HOLD):
                            pool_c, tag_c = srcs[c]
                            pst = pool_c.tile(
                                [P, MB], F32, tag=tag_c, name="pst_tail"
                            )
                            mi, nh = divmod(c, 2)
                            for cc in range(KC - 2):
                                nc.tensor.matmul(
                                    pst[:],
                                    ot_t[:, cc, mi * P : (mi + 1) * P],
                                    wo_sb[:, cc, nh * MB : (nh + 1) * MB],
                                    start=(cc == 0),
                                    stop=False,
                                )
                            held.append(pst)
                        for c, pst in enumerate(held):
                            mi, nh = divmod(c, 2)
                            for cc in (KC - 2, KC - 1):
                                nc.tensor.matmul(
                                    pst[:],
                                    ot_t[:, cc, mi * P : (mi + 1) * P],
                                    wo_sb[:, cc, nh * MB : (nh + 1) * MB],
                                    start=False,
                                    stop=(cc == KC - 1),
                                )
                            oproj_finish(j, ot_t, c, pst)
                        for c in range(HOLD, 2 * MSUB):
                            pst = oproj_matmuls(j, ot_t, c)
                            oproj_finish(
                                j, ot_t, c, pst, split_store=(c >= 2 * MSUB - 2)
                            )

                    for ji, j in enumerate(range(NBLK)):
                        # prefetch next block's input; for block 0 the issue
                        # is deferred into the pair loop (with wo) to keep
                        # the startup window's HBM bandwidth for the weights
                        # the prologue is actually waiting on
                        xt_next = None
                        if j + 1 < NBLK:
                            xt_next = xt_pool.tile(
                                [P, KC, MB], BF16, tag="xt", name="xt_t"
                            )
                            if ji != 0:
                                nc.sync.dma_start(
                                    xt_next[:],
                                    xt_v[:, :, (j + 1) * MB : (j + 2) * MB],
                                )
                        qt_t = qt_first if ji == 0 else qt_next  # noqa: F821
                        qt_next = None

                        # normalized O^T for this block, written per-pair
                        ot_t = ot_pool.tile([P, KC, MB], BF16, tag="ot2", name="ot_t")

                        for jc in range(KC):
                            pss2 = ps_s.tile([T, 2, MB], F32, tag="pss")
                            for hf in range(2):
                                nc.tensor.matmul(
                                    pss2[:, hf, :],
                                    kt_sb[64 * hf : 64 * hf + 64, jc, :],
                                    qt_t[64 * hf : 64 * hf + 64, jc, :],
                                    start=True,
                                    stop=True,
                                )
                            es2 = es_pool.tile([T, 2, MB], BF16, tag="es")
                            nc.scalar.activation(
                                es2[:], pss2[:], AF.Exp, scale=SCALE
                            )
                            pso2 = ps_o.tile([D + 1, 2, MB], F32, tag="pso")
                            for hf in range(2):
                                nc.tensor.matmul(
                                    pso2[:, hf, :],
                                    v_sb[:, 2 * jc + hf, :],
                                    es2[:, hf, :],
                                    start=True,
                                    stop=True,
                                )
                            # pair's unnormalized head outputs + denom rows
                            oud_j = ou_pool.tile(
                                [D + 1, 2, MB], BF16, tag="ou", name="oud_j"
                            )
                            tail_pair = (
                                rep == repeat - 1
                                and j == NBLK - 1
                                and jc == KC - 1
                            )
                            if tail_pair:
                                # final pair: copy halves on ACT and DVE in
                                # parallel so the epilogue chain is short
                                nc.scalar.copy(oud_j[:, 0, :], pso2[:, 0, :])
                                nc.vector.tensor_copy(
                                    out=oud_j[:, 1, :], in_=pso2[:, 1, :]
                                )
                            elif jc % 2 == 0:
                                nc.scalar.copy(oud_j[:], pso2[:])
                            else:
                                nc.vector.tensor_copy(out=oud_j[:], in_=pso2[:])

                            # per-pair normalize: gather denom rows, recip,
                            # broadcast, apply.  Chain is hidden under the
                            # PE's per-pair work; the hf=0 multiply runs on
                            # the otherwise-idle GpSimd except for the final
                            # pair, where DVE keeps the epilogue chain short.
                            def emit_norm(jc, oud_j, tail_pair):
                                dn_j = dn_pool.tile(
                                    [2, MB], BF16, tag="dn", name="dn_j"
                                )
                                nc.sync.dma_start(
                                    dn_j[:], oud_j[D : D + 1, :, :]
                                )
                                rc_j = rc_pool.tile(
                                    [2, MB], BF16, tag="rc", name="rc_j"
                                )
                                with nc.allow_low_precision(
                                    reason="softmax recip; rel tol 2e-2"
                                ):
                                    nc.vector.reciprocal(rc_j[:], dn_j[:])
                                rb_j = rb_pool.tile(
                                    [D, 2, MB], BF16, tag="rb", name="rb_j"
                                )
                                for hf in range(2):
                                    nc.sync.dma_start(
                                        rb_j[:, hf, :],
                                        rc_j[hf : hf + 1, None, :].to_broadcast(
                                            (1, D, MB)
                                        ),
                                    )
                                eng0 = nc.vector if tail_pair else nc.gpsimd
                                eng0.tensor_tensor(
                                    ot_t[0:64, jc, :],
                                    oud_j[0:D, 0, :],
                                    rb_j[:, 0, :],
                                    mybir.AluOpType.mult,
                                )
                                eng0.tensor_tensor(
                                    ot_t[64:128, jc, :],
                                    oud_j[0:D, 1, :],
                                    rb_j[:, 1, :],
                                    mybir.AluOpType.mult,
                                )

                            # the LAST pair's normalize is emitted after the
                            # next block's Q-projection: its recip/multiply
                            # carry DMA waits that would head-of-line block
                            # the psq copies behind them in the DVE queue
                            if jc < KC - 1 or j + 1 >= NBLK:
                                emit_norm(jc, oud_j, tail_pair)
                                norm_pending = None
                            else:
                                norm_pending = (jc, oud_j)

                            # previous block, deferred: one O-projection
                            # chunk per pair
                            if prev is not None:
                                emit_oproj_chunk(prev[0], prev[1], jc)

                            # block 0: late-issued loads that would other-
                            # wise crowd the startup bandwidth window
                            if ji == 0:
                                if jc == 1:
                                    nc.scalar.dma_start(wo_sb[:], wo_v[:])
                                if jc == 3 and xt_next is not None:
                                    nc.sync.dma_start(
                                        xt_next[:],
                                        xt_v[:, :, (j + 1) * MB : (j + 2) * MB],
                                    )

                        if j + 1 < NBLK:
                            qt_next = emit_qproj(j + 1, xt_next)
                            if norm_pending is not None:
                                emit_norm(norm_pending[0], norm_pending[1], False)
                        prev = (j, ot_t)

                # epilogue: final repetition's last block
                emit_oproj_tail(prev[0], prev[1])
    nc.finalize()
    return nc


def _get_nc(repeat: int = 1):
    global _CACHED_NC
    if _CACHED_NC is None:
        _CACHED_NC = {}
    if repeat not in _CACHED_NC:
        _CACHED_NC[repeat] = _build(repeat)
    return _CACHED_NC[repeat]


def _bf16(a):
    import ml_dtypes

    return np.asarray(a, dtype=ml_dtypes.bfloat16)


def kernel(video_features, text_features, Wq, Wk, Wv, Wo, **_unused):
    video_features = np.asarray(video_features, dtype=np.float32)
    text_features = np.asarray(text_features, dtype=np.float32)
    wqt = _bf16(np.asarray(Wq, dtype=np.float32).T)
    wkt = _bf16(np.asarray(Wk, dtype=np.float32).T)
    wvt = _bf16(np.asarray(Wv, dtype=np.float32).T)
    wot = _bf16(np.asarray(Wo, dtype=np.float32).T)

    in_maps = []
    for c in range(8):
        b, half = divmod(c, 2)
        xs = video_features[b, half * M : (half + 1) * M, :]  # [M, C]
        in_maps.append(
            {
                "xt": _bf16(np.ascontiguousarray(xs.T)),
                "yt": _bf16(np.ascontiguousarray(text_features[b].T)),
                "wqt": wqt,
                "wkt": wkt,
                "wvt": wvt,
                "wot": wot,
            }
        )

    res = run_bass_kernel_spmd(_get_nc(), in_maps, core_ids=list(range(8)))
    outf = np.empty((B, T_V, C), dtype=np.float32)
    for c in range(8):
        b, half = divmod(c, 2)
        outf[b, half * M : (half + 1) * M, :] = np.asarray(
            res.results[c]["out"], dtype=np.float32
        )
    return outf


# revision 53
# speedup vs baseline: 1.1286x; 1.0552x over previous
"""Cross-attention kernel for 8 TRN2 NeuronCores.

Problem: B=4, T_V=8192, T_T=77, C=1024, H=16, D=64.
  q = video @ Wq.T ; k,v = text @ W.T ; out = softmax(qk/sqrt(D)) v @ Wo.T

Sharding: data-parallel over (batch, T_V/2) -> 8 shards of [4096, 1024].

Structure (~25% over the previous 415us baseline, sim 310us):
  - bf16 on the wire and in SBUF, f32 PSUM accumulation.
  - head-PAIR attention: two S / two AV matmuls feed one 2-bank PSUM
    tile, so exp and the PSUM->SBUF copy run once per pair.
  - PER-PAIR softmax normalization: as soon as a pair's [O^T | denom]
    lands in SBUF, its denominator rows are gathered, reciprocal'd,
    partition-broadcast (2 small DMAs on the SP HW queue) and applied
    by two [64, MB] multiplies — normalized O^T (ot_t) for block j is
    complete a few us after its last AV matmul instead of ~17 us.
    This removes the PE head-of-line stall at every block boundary
    (the deferred O-projection no longer waits on a late normalize
    chain) and shrinks the epilogue tail.  Both multiplies run on
    the otherwise-idle GpSimd (a measured ~25us/call win on HW over
    putting them on DVE); the final pair stays on DVE for chain
    latency.  The last pair's normalize is emitted after the next
    block's Q-projection so its DMA waits can't head-of-line block
    the psq copies in the DVE queue.
  - block j's O-projection is deferred into block j+1 (one chunk per
    pair), carried across `repeat` boundaries, so the PE never drains.
  - epilogue: six O-proj chunks borrow the dead psq/pso/pss PSUM
    slots and hold back their cc=6,7 accumulations, queueing ~7.7us
    of PE work while the final pairs' normalize chains complete; the
    last row-slab stores split across both HW queues.
  - startup: xt/Wq split across the SP and ACT HW DGE queues so the
    first Q-proj matmul group starts after ~1.25 MB of parallel
    transfers; wo and the block-1 xt prefetch issue from inside block
    0's pair loop to keep the bandwidth-bound startup window clear.
  - Q-proj PSUM->SBUF copies and pair copies alternate ScalarE/
    VectorE; output-slab stores ride the Activation HW queue right
    behind their ob copies; merged [128, 1024] output DMAs.
  - 4-deep SBUF rings on the per-pair tiles and the steady-state xt
    prefetch split across both HW DGE queues: a further measured
    ~21us/call on HW (scheduling slack the cost model undervalues).
"""

import sys

if "/opt/trn_rl_repo" not in sys.path:
    sys.path.insert(0, "/opt/trn_rl_repo")

import numpy as np

from contextlib import ExitStack

import concourse.bacc as bacc
import concourse.mybir as mybir
import concourse.tile as tile
from concourse.bass_utils import run_bass_kernel_spmd

F32 = mybir.dt.float32
BF16 = mybir.dt.bfloat16
AF = mybir.ActivationFunctionType

B, T_V, T_T, C, H = 4, 8192, 77, 1024, 16
D = C // H            # 64
P = 128
KC = C // P           # 8 contraction chunks
M = T_V // 2          # 4096 rows per core
MB = 512              # m-block
NBLK = M // MB        # 8
MSUB = MB // P        # 4
T = T_T               # 77
TP = 80
SCALE = 1.0 / float(np.sqrt(D))

_CACHED_NC = None


def _build(repeat: int = 1):
    nc = bacc.Bacc(name="cross_attention")

    xt = nc.dram_tensor("xt", [C, M], BF16, kind="ExternalInput")
    yt = nc.dram_tensor("yt", [C, T], BF16, kind="ExternalInput")
    wqt = nc.dram_tensor("wqt", [C, C], BF16, kind="ExternalInput")
    wkt = nc.dram_tensor("wkt", [C, C], BF16, kind="ExternalInput")
    wvt = nc.dram_tensor("wvt", [C, C], BF16, kind="ExternalInput")
    wot = nc.dram_tensor("wot", [C, C], BF16, kind="ExternalInput")
    out = nc.dram_tensor("out", [M, C], BF16, kind="ExternalOutput")

    xt_v = xt[:, :].rearrange("(kc p) m -> p kc m", p=P)
    yt_v = yt[:, :].rearrange("(kc p) t -> p kc t", p=P)
    wq_v = wqt[:, :].rearrange("(kc p) n -> p kc n", p=P)
    wk_v = wkt[:, :].rearrange("(kc p) n -> p kc n", p=P)
    wv_v = wvt[:, :].rearrange("(kc p) n -> p kc n", p=P)
    wo_v = wot[:, :].rearrange("(kc p) n -> p kc n", p=P)

    with tile.TileContext(nc) as tc:
        with (
            tc.tile_pool(name="wq", bufs=2) as wq_pool,
            tc.tile_pool(name="wo", bufs=2) as wo_pool,
            tc.tile_pool(name="kt", bufs=2) as kt_pool,
            tc.tile_pool(name="vv", bufs=2) as v_pool,
        ):

            # ---- main pipeline (prologue nested; psq pool shared) ----
            with ExitStack() as stack:
                pool = lambda *a, **k: stack.enter_context(tc.tile_pool(*a, **k))
                xt_pool = pool(name="xt", bufs=2)
                qt_pool = pool(name="qt", bufs=2)
                ot_pool = pool(name="ot", bufs=2)
                es_pool = pool(name="es", bufs=4)
                ou_pool = pool(name="ou", bufs=4)
                dn_pool = pool(name="dn", bufs=4)
                rc_pool = pool(name="rc", bufs=4)
                rb_pool = pool(name="rb", bufs=4)
                ob_pool = pool(name="ob", bufs=4)
                ps_q = pool(name="psq", bufs=2, space="PSUM")
                ps_s = pool(name="pss", bufs=1, space="PSUM")
                ps_o = pool(name="pso", bufs=1, space="PSUM")
                ps_out = pool(name="psout", bufs=2, space="PSUM")

                prev = None  # (j, ot_t) pending O-proj
                for rep in range(repeat):
                    wq_sb = wq_pool.tile([P, KC, C], BF16, tag="wq", name="wq_sb")
                    wo_sb = wo_pool.tile([P, KC, C], BF16, tag="wo", name="wo_sb")
                    kt_sb = kt_pool.tile([P, KC, T], BF16, tag="kt", name="kt_sb")
                    v_sb = v_pool.tile([T, H, D + 1], BF16, tag="vv", name="v_sb")
                    # startup prefetches: block-0 Q-proj dependencies first,
                    # split across both HW DGE queues (SP + ACT) so the
                    # transfers overlap: the first matmul group waits on two
                    # parallel ~0.5 MB xt halves plus one Wq n-chunk; the
                    # other Wq chunks stream at the rate the PE consumes
                    # them.  K weights follow on SP, V/O weights on ACT.
                    xt_first = xt_pool.tile([P, KC, MB], BF16, tag="xt", name="xt_t")
                    nc.sync.dma_start(
                        xt_first[:, : KC // 2, :], xt_v[:, : KC // 2, 0:MB]
                    )
                    nc.scalar.dma_start(wq_sb[:, :, 0:P], wq_v[:, :, 0:P])
                    nc.scalar.dma_start(
                        xt_first[:, KC // 2 :, :], xt_v[:, KC // 2 :, 0:MB]
                    )
                    nc.sync.dma_start(
                        wq_sb[:, :, P : P * KC // 2],
                        wq_v[:, :, P : P * KC // 2],
                    )
                    nc.scalar.dma_start(
                        wq_sb[:, :, P * KC // 2 :],
                        wq_v[:, :, P * KC // 2 :],
                    )
                    yt_sb = es_pool.tile([P, KC, TP], BF16, tag="es", name="yt_sb")
                    nc.vector.memset(yt_sb[:], 0.0)
                    nc.sync.dma_start(yt_sb[:, :, :T], yt_v[:])

                    def emit_qproj_group(qt_t, xt_t, nc_):
                        psq = ps_q.tile([P, MB], F32, tag="psq", name="psq")
                        for kc in range(KC):
                            nc.tensor.matmul(
                                psq[:],
                                wq_sb[:, kc, nc_ * P : (nc_ + 1) * P],
                                xt_t[:, kc, :],
                                start=(kc == 0),
                                stop=(kc == KC - 1),
                            )
                        # PSUM->SBUF copies alternate ACT/DVE so the psq
                        # ring never gates on one busy copy engine
                        if nc_ % 2 == 0:
                            nc.scalar.copy(qt_t[:, nc_, :], psq[:])
                        else:
                            nc.vector.tensor_copy(out=qt_t[:, nc_, :], in_=psq[:])

                    def emit_qproj(j, xt_t):
                        qt_t = qt_pool.tile([P, KC, MB], BF16, tag="qt", name="qt_t")
                        for nc_ in range(KC):
                            emit_qproj_group(qt_t, xt_t, nc_)
                        return qt_t

                    qt_first = emit_qproj(0, xt_first)

                    # ---- prologue: K^T and V from text (PSUM via psq ring) ----
                    wk_sb = ot_pool.tile([P, KC, C], BF16, tag="oud", name="wk_sb")
                    for kc in range(KC):
                        nc.sync.dma_start(wk_sb[:, kc, :], wk_v[:, kc, :])
                    for nc_ in range(KC):
                        psk_full = ps_q.tile([P, MB], F32, tag="psq", name="psk")
                        psk = psk_full[:, :TP]
                        for kc in range(KC):
                            nc.tensor.matmul(
                                psk[:],
                                wk_sb[:, kc, nc_ * P : (nc_ + 1) * P],
                                yt_sb[:, kc, :],
                                start=(kc == 0),
                                stop=(kc == KC - 1),
                            )
                        nc.vector.tensor_copy(out=kt_sb[:, nc_, :], in_=psk[:, :T])

                    wv_sb = ot_pool.tile([P, KC, C], BF16, tag="oud", name="wv_sb")
                    nc.sync.dma_start(wv_sb[:], wv_v[:])
                    for half in range(2):
                        psv_full = ps_q.tile([P, MB], F32, tag="psq", name="psv")
                        psv = psv_full[:T, :]
                        for kc in range(KC):
                            nc.tensor.matmul(
                                psv[:],
                                yt_sb[:, kc, :T],
                                wv_sb[:, kc, half * MB : (half + 1) * MB],
                                start=(kc == 0),
                                stop=(kc == KC - 1),
                            )
                        nc.vector.tensor_copy(
                            out=v_sb[:, half * 8 : (half + 1) * 8, 0:D],
                            in_=psv[:].rearrange("t (h d) -> t h d", d=D),
                        )
                    nc.vector.memset(v_sb[:, :, D : D + 1], 1.0)
                    # wo is first needed when block 0's O-projection runs
                    # (during block 1): its load is issued from inside block
                    # 0's pair loop so the 2 MB transfer stays out of the
                    # bandwidth-bound startup window

                    ob_cur = [None]

                    def oproj_matmuls(j, ot_t, chunk, cc_hi=KC):
                        mi, nh = divmod(chunk, 2)
                        pst = ps_out.tile([P, MB], F32, tag="psout")
                        for cc in range(cc_hi):
                            nc.tensor.matmul(
                                pst[:],
                                ot_t[:, cc, mi * P : (mi + 1) * P],
                                wo_sb[:, cc, nh * MB : (nh + 1) * MB],
                                start=(cc == 0),
                                stop=(cc == KC - 1),
                            )
                        return pst

                    def oproj_finish(j, ot_t, chunk, pst, split_store=False):
                        # row-slab [128, 1024] stores go out as one DMA on the
                        # Activation HW queue (its ob copy just ran there, so
                        # the trigger issues with no wait and no head-of-line
                        # blocking of attention-critical DMAs).  split_store
                        # issues two half-slab stores so the final output DMA
                        # overlaps the last chunk's matmuls.
                        mi, nh = divmod(chunk, 2)
                        if nh == 0:
                            ob_cur[0] = ob_pool.tile(
                                [P, C], BF16, tag="ob", name="ob"
                            )
                        ob = ob_cur[0]
                        on_dve = (nh == 0) if split_store else (mi % 2 == 0)
                        if on_dve:
                            nc.vector.tensor_copy(
                                out=ob[:, nh * MB : (nh + 1) * MB], in_=pst[:]
                            )
                        else:
                            nc.scalar.copy(ob[:, nh * MB : (nh + 1) * MB], pst[:])
                        rows = slice(j * MB + mi * P, j * MB + (mi + 1) * P)
                        if split_store:
                            # half-slab stores on separate HW queues so the
                            # final two transfers overlap
                            eng = nc.sync if nh == 0 else nc.scalar
                            eng.dma_start(
                                out[rows, nh * MB : (nh + 1) * MB],
                                ob[:, nh * MB : (nh + 1) * MB],
                            )
                        elif nh == 1:
                            nc.scalar.dma_start(out[rows, :], ob[:])

                    def emit_oproj_chunk(j, ot_t, chunk):
                        pst = oproj_matmuls(j, ot_t, chunk)
                        oproj_finish(j, ot_t, chunk, pst)

                    def emit_oproj_tail(j, ot_t):
                        # epilogue: six chunks accumulate cc=0..5 first —
                        # borrowing the now-dead psq/pss/pso PSUM slots (same
                        # tags, compatible sizes) for ~7.7us of queued PE work
                        # — so the last two pairs' normalize chains complete
                        # before any chunk needs their cc=6/7 accumulations.
                        HOLD = 6
                        srcs = [
                            (ps_out, "psout"),
                            (ps_out, "psout"),
                            (ps_q, "psq"),
                            (ps_q, "psq"),
                            (ps_o, "pso"),
                            (ps_s, "pss"),
                        ]
                        held = []
                        for c in range(HOLD):
                            pool_c, tag_c = srcs[c]
                            pst = pool_c.tile(
                                [P, MB], F32, tag=tag_c, name="pst_tail"
                            )
                            mi, nh = divmod(c, 2)
                            for cc in range(KC - 2):
                                nc.tensor.matmul(
                                    pst[:],
                                    ot_t[:, cc, mi * P : (mi + 1) * P],
                                    wo_sb[:, cc, nh * MB : (nh + 1) * MB],
                                    start=(cc == 0),
                                    stop=False,
                                )
                            held.append(pst)
                        for c, pst in enumerate(held):
                            mi, nh = divmod(c, 2)
                            for cc in (KC - 2, KC - 1):
                                nc.tensor.matmul(
                                    pst[:],
                                    ot_t[:, cc, mi * P : (mi + 1) * P],
                                    wo_sb[:, cc, nh * MB : (nh + 1) * MB],
                                    start=False,
                                    stop=(cc == KC - 1),
                                )
                            oproj_finish(j, ot_t, c, pst)
                        for c in range(HOLD, 2 * MSUB):
                            pst = oproj_matmuls(j, ot_t, c)
                            oproj_finish(
                                j, ot_t, c, pst, split_store=(c >= 2 * MSUB - 2)
                            )

                    for ji, j in enumerate(range(NBLK)):
                        # prefetch next block's input; for block 0 the issue
                        # is deferred into the pair loop (with wo) to keep
                        # the startup window's HBM bandwidth for the weights
                        # the prologue is actually waiting on
                        xt_next = None
                        if j + 1 < NBLK:
                            xt_next = xt_pool.tile(
                                [P, KC, MB], BF16, tag="xt", name="xt_t"
                            )
                            if ji != 0:
                                nc.sync.dma_start(
                                    xt_next[:, : KC // 2, :],
                                    xt_v[:, : KC // 2, (j + 1) * MB : (j + 2) * MB],
                                )
                                nc.scalar.dma_start(
                                    xt_next[:, KC // 2 :, :],
                                    xt_v[:, KC // 2 :, (j + 1) * MB : (j + 2) * MB],
                                )
                        qt_t = qt_first if ji == 0 else qt_next  # noqa: F821
                        qt_next = None

                        # normalized O^T for this block, written per-pair
                        ot_t = ot_pool.tile([P, KC, MB], BF16, tag="ot2", name="ot_t")

                        for jc in range(KC):
                            pss2 = ps_s.tile([T, 2, MB], F32, tag="pss")
                            for hf in range(2):
                                nc.tensor.matmul(
                                    pss2[:, hf, :],
                                    kt_sb[64 * hf : 64 * hf + 64, jc, :],
                                    qt_t[64 * hf : 64 * hf + 64, jc, :],
                                    start=True,
                                    stop=True,
                                )
                            es2 = es_pool.tile([T, 2, MB], BF16, tag="es")
                            nc.scalar.activation(
                                es2[:], pss2[:], AF.Exp, scale=SCALE
                            )
                            pso2 = ps_o.tile([D + 1, 2, MB], F32, tag="pso")
                            for hf in range(2):
                                nc.tensor.matmul(
                                    pso2[:, hf, :],
                                    v_sb[:, 2 * jc + hf, :],
                                    es2[:, hf, :],
                                    start=True,
                                    stop=True,
                                )
                            # pair's unnormalized head outputs + denom rows
                            oud_j = ou_pool.tile(
                                [D + 1, 2, MB], BF16, tag="ou", name="oud_j"
                            )
                            tail_pair = (
                                rep == repeat - 1
                                and j == NBLK - 1
                                and jc == KC - 1
                            )
                            if tail_pair:
                                # final pair: copy halves on ACT and DVE in
                                # parallel so the epilogue chain is short
                                nc.scalar.copy(oud_j[:, 0, :], pso2[:, 0, :])
                                nc.vector.tensor_copy(
                                    out=oud_j[:, 1, :], in_=pso2[:, 1, :]
                                )
                            elif jc % 2 == 0:
                                nc.scalar.copy(oud_j[:], pso2[:])
                            else:
                                nc.vector.tensor_copy(out=oud_j[:], in_=pso2[:])

                            # per-pair normalize: gather denom rows, recip,
                            # broadcast, apply.  Chain is hidden under the
                            # PE's per-pair work; the hf=0 multiply runs on
                            # the otherwise-idle GpSimd except for the final
                            # pair, where DVE keeps the epilogue chain short.
                            def emit_norm(jc, oud_j, tail_pair):
                                dn_j = dn_pool.tile(
                                    [2, MB], BF16, tag="dn", name="dn_j"
                                )
                                nc.sync.dma_start(
                                    dn_j[:], oud_j[D : D + 1, :, :]
                                )
                                rc_j = rc_pool.tile(
                                    [2, MB], BF16, tag="rc", name="rc_j"
                                )
                                with nc.allow_low_precision(
                                    reason="softmax recip; rel tol 2e-2"
                                ):
                                    nc.vector.reciprocal(rc_j[:], dn_j[:])
                                rb_j = rb_pool.tile(
                                    [D, 2, MB], BF16, tag="rb", name="rb_j"
                                )
                                for hf in range(2):
                                    nc.sync.dma_start(
                                        rb_j[:, hf, :],
                                        rc_j[hf : hf + 1, None, :].to_broadcast(
                                            (1, D, MB)
                                        ),
                                    )
                                eng0 = nc.vector if tail_pair else nc.gpsimd
                                eng0.tensor_tensor(
                                    ot_t[0:64, jc, :],
                                    oud_j[0:D, 0, :],
                                    rb_j[:, 0, :],
                                    mybir.AluOpType.mult,
                                )
                                eng0.tensor_tensor(
                                    ot_t[64:128, jc, :],
                                    oud_j[0:D, 1, :],
                                    rb_j[:, 1, :],
                                    mybir.AluOpType.mult,
                                )

                            # the LAST pair's normalize is emitted after the
                            # next block's Q-projection: its recip/multiply
                            # carry DMA waits that would head-of-line block
                            # the psq copies behind them in the DVE queue
                            if jc < KC - 1 or j + 1 >= NBLK:
                                emit_norm(jc, oud_j, tail_pair)
                                norm_pending = None
                            else:
                                norm_pending = (jc, oud_j)

                            # previous block, deferred: one O-projection
                            # chunk per pair
                            if prev is not None:
                                emit_oproj_chunk(prev[0], prev[1], jc)

                            # block 0: late-issued loads that would other-
                            # wise crowd the startup bandwidth window
                            if ji == 0:
                                if jc == 1:
                                    nc.scalar.dma_start(wo_sb[:], wo_v[:])
                                if jc == 3 and xt_next is not None:
                                    nc.sync.dma_start(
                                        xt_next[:],
                                        xt_v[:, :, (j + 1) * MB : (j + 2) * MB],
                                    )

                        if j + 1 < NBLK:
                            qt_next = emit_qproj(j + 1, xt_next)
                            if norm_pending is not None:
                                emit_norm(norm_pending[0], norm_pending[1], False)
                        prev = (j, ot_t)

                # epilogue: final repetition's last block
                emit_oproj_tail(prev[0], prev[1])
    nc.finalize()
    return nc


def _get_nc(repeat: int = 1):
    global _CACHED_NC
    if _CACHED_NC is None:
        _CACHED_NC = {}
    if repeat not in _CACHED_NC:
        _CACHED_NC[repeat] = _build(repeat)
    return _CACHED_NC[repeat]


def _bf16(a):
    import ml_dtypes

    return np.asarray(a, dtype=ml_dtypes.bfloat16)


def kernel(video_features, text_features, Wq, Wk, Wv, Wo, **_unused):
    video_features = np.asarray(video_features, dtype=np.float32)
    text_features = np.asarray(text_features, dtype=np.float32)
    wqt = _bf16(np.asarray(Wq, dtype=np.float32).T)
    wkt = _bf16(np.asarray(Wk, dtype=np.float32).T)
    wvt = _bf16(np.asarray(Wv, dtype=np.float32).T)
    wot = _bf16(np.asarray(Wo, dtype=np.float32).T)

    in_maps = []
    for c in range(8):
        b, half = divmod(c, 2)
        xs = video_features[b, half * M : (half + 1) * M, :]  # [M, C]
        in_maps.append(
            {
                "xt": _bf16(np.ascontiguousarray(xs.T)),
                "yt": _bf16(np.ascontiguousarray(text_features[b].T)),
                "wqt": wqt,
                "wkt": wkt,
                "wvt": wvt,
                "wot": wot,
            }
        )

    res = run_bass_kernel_spmd(_get_nc(), in_maps, core_ids=list(range(8)))
    outf = np.empty((B, T_V, C), dtype=np.float32)
    for c in range(8):
        b, half = divmod(c, 2)
        outf[b, half * M : (half + 1) * M, :] = np.asarray(
            res.results[c]["out"], dtype=np.float32
        )
    return outf


# revision 55
# speedup vs baseline: 1.1703x; 1.0369x over previous
"""Cross-attention kernel for 8 TRN2 NeuronCores.

Problem: B=4, T_V=8192, T_T=77, C=1024, H=16, D=64.
  q = video @ Wq.T ; k,v = text @ W.T ; out = softmax(qk/sqrt(D)) v @ Wo.T

Sharding: data-parallel over (batch, T_V/2) -> 8 shards of [4096, 1024].

Structure (~25% over the previous 415us baseline, sim 310us):
  - bf16 on the wire and in SBUF, f32 PSUM accumulation.
  - head-PAIR attention: two S / two AV matmuls feed one 2-bank PSUM
    tile, so exp and the PSUM->SBUF copy run once per pair.
  - PER-PAIR softmax normalization: as soon as a pair's [O^T | denom]
    lands in SBUF, its denominator rows are gathered, reciprocal'd,
    partition-broadcast (2 small DMAs on the SP HW queue) and applied
    by two [64, MB] multiplies — normalized O^T (ot_t) for block j is
    complete a few us after its last AV matmul instead of ~17 us.
    This removes the PE head-of-line stall at every block boundary
    (the deferred O-projection no longer waits on a late normalize
    chain) and shrinks the epilogue tail.  Both multiplies run on
    the otherwise-idle GpSimd (a measured ~25us/call win on HW over
    putting them on DVE); the final pair stays on DVE for chain
    latency.  The last pair's normalize is emitted after the next
    block's Q-projection so its DMA waits can't head-of-line block
    the psq copies in the DVE queue.
  - block j's O-projection is deferred into block j+1 (one chunk per
    pair), carried across `repeat` boundaries, so the PE never drains.
  - epilogue: six O-proj chunks borrow the dead psq/pso/pss PSUM
    slots and hold back their cc=6,7 accumulations, queueing ~7.7us
    of PE work while the final pairs' normalize chains complete; the
    last row-slab stores split across both HW queues.
  - startup: xt/Wq split across the SP and ACT HW DGE queues so the
    first Q-proj matmul group starts after ~1.25 MB of parallel
    transfers; wo and the block-1 xt prefetch issue from inside block
    0's pair loop to keep the bandwidth-bound startup window clear.
  - Q-proj PSUM->SBUF copies and pair copies alternate ScalarE/
    VectorE; output-slab stores ride the Activation HW queue right
    behind their ob copies; merged [128, 1024] output DMAs.
  - 4-deep SBUF rings on the per-pair tiles and the steady-state xt
    prefetch split across both HW DGE queues: a further measured
    ~21us/call on HW (scheduling slack the cost model undervalues).
  - even-pair denominator gathers ride the ACT HW queue right behind
    the pair copy that just ran there (zero-wait trigger, halves the
    SP queue's per-pair load) and ot_t uses a 3-deep ring: a further
    measured ~30us/call on HW.
"""

import sys

if "/opt/trn_rl_repo" not in sys.path:
    sys.path.insert(0, "/opt/trn_rl_repo")

import numpy as np

from contextlib import ExitStack

import concourse.bacc as bacc
import concourse.mybir as mybir
import concourse.tile as tile
from concourse.bass_utils import run_bass_kernel_spmd

F32 = mybir.dt.float32
BF16 = mybir.dt.bfloat16
AF = mybir.ActivationFunctionType

B, T_V, T_T, C, H = 4, 8192, 77, 1024, 16
D = C // H            # 64
P = 128
KC = C // P           # 8 contraction chunks
M = T_V // 2          # 4096 rows per core
MB = 512              # m-block
NBLK = M // MB        # 8
MSUB = MB // P        # 4
T = T_T               # 77
TP = 80
SCALE = 1.0 / float(np.sqrt(D))

_CACHED_NC = None


def _build(repeat: int = 1):
    nc = bacc.Bacc(name="cross_attention")

    xt = nc.dram_tensor("xt", [C, M], BF16, kind="ExternalInput")
    yt = nc.dram_tensor("yt", [C, T], BF16, kind="ExternalInput")
    wqt = nc.dram_tensor("wqt", [C, C], BF16, kind="ExternalInput")
    wkt = nc.dram_tensor("wkt", [C, C], BF16, kind="ExternalInput")
    wvt = nc.dram_tensor("wvt", [C, C], BF16, kind="ExternalInput")
    wot = nc.dram_tensor("wot", [C, C], BF16, kind="ExternalInput")
    out = nc.dram_tensor("out", [M, C], BF16, kind="ExternalOutput")

    xt_v = xt[:, :].rearrange("(kc p) m -> p kc m", p=P)
    yt_v = yt[:, :].rearrange("(kc p) t -> p kc t", p=P)
    wq_v = wqt[:, :].rearrange("(kc p) n -> p kc n", p=P)
    wk_v = wkt[:, :].rearrange("(kc p) n -> p kc n", p=P)
    wv_v = wvt[:, :].rearrange("(kc p) n -> p kc n", p=P)
    wo_v = wot[:, :].rearrange("(kc p) n -> p kc n", p=P)

    with tile.TileContext(nc) as tc:
        with (
            tc.tile_pool(name="wq", bufs=2) as wq_pool,
            tc.tile_pool(name="wo", bufs=2) as wo_pool,
            tc.tile_pool(name="kt", bufs=2) as kt_pool,
            tc.tile_pool(name="vv", bufs=2) as v_pool,
        ):

            # ---- main pipeline (prologue nested; psq pool shared) ----
            with ExitStack() as stack:
                pool = lambda *a, **k: stack.enter_context(tc.tile_pool(*a, **k))
                xt_pool = pool(name="xt", bufs=2)
                qt_pool = pool(name="qt", bufs=2)
                ot_pool = pool(name="ot", bufs=2)
                es_pool = pool(name="es", bufs=4)
                ou_pool = pool(name="ou", bufs=4)
                dn_pool = pool(name="dn", bufs=4)
                rc_pool = pool(name="rc", bufs=4)
                rb_pool = pool(name="rb", bufs=4)
                ob_pool = pool(name="ob", bufs=4)
                ps_q = pool(name="psq", bufs=2, space="PSUM")
                ps_s = pool(name="pss", bufs=1, space="PSUM")
                ps_o = pool(name="pso", bufs=1, space="PSUM")
                ps_out = pool(name="psout", bufs=2, space="PSUM")

                prev = None  # (j, ot_t) pending O-proj
                for rep in range(repeat):
                    wq_sb = wq_pool.tile([P, KC, C], BF16, tag="wq", name="wq_sb")
                    wo_sb = wo_pool.tile([P, KC, C], BF16, tag="wo", name="wo_sb")
                    kt_sb = kt_pool.tile([P, KC, T], BF16, tag="kt", name="kt_sb")
                    v_sb = v_pool.tile([T, H, D + 1], BF16, tag="vv", name="v_sb")
                    # startup prefetches: block-0 Q-proj dependencies first,
                    # split across both HW DGE queues (SP + ACT) so the
                    # transfers overlap: the first matmul group waits on two
                    # parallel ~0.5 MB xt halves plus one Wq n-chunk; the
                    # other Wq chunks stream at the rate the PE consumes
                    # them.  K weights follow on SP, V/O weights on ACT.
                    xt_first = xt_pool.tile([P, KC, MB], BF16, tag="xt", name="xt_t")
                    nc.sync.dma_start(
                        xt_first[:, : KC // 2, :], xt_v[:, : KC // 2, 0:MB]
                    )
                    nc.scalar.dma_start(wq_sb[:, :, 0:P], wq_v[:, :, 0:P])
                    nc.scalar.dma_start(
                        xt_first[:, KC // 2 :, :], xt_v[:, KC // 2 :, 0:MB]
                    )
                    nc.sync.dma_start(
                        wq_sb[:, :, P : P * KC // 2],
                        wq_v[:, :, P : P * KC // 2],
                    )
                    nc.scalar.dma_start(
                        wq_sb[:, :, P * KC // 2 :],
                        wq_v[:, :, P * KC // 2 :],
                    )
                    yt_sb = es_pool.tile([P, KC, TP], BF16, tag="es", name="yt_sb")
                    nc.vector.memset(yt_sb[:], 0.0)
                    nc.sync.dma_start(yt_sb[:, :, :T], yt_v[:])

                    def emit_qproj_group(qt_t, xt_t, nc_):
                        psq = ps_q.tile([P, MB], F32, tag="psq", name="psq")
                        for kc in range(KC):
                            nc.tensor.matmul(
                                psq[:],
                                wq_sb[:, kc, nc_ * P : (nc_ + 1) * P],
                                xt_t[:, kc, :],
                                start=(kc == 0),
                                stop=(kc == KC - 1),
                            )
                        # PSUM->SBUF copies alternate ACT/DVE so the psq
                        # ring never gates on one busy copy engine
                        if nc_ % 2 == 0:
                            nc.scalar.copy(qt_t[:, nc_, :], psq[:])
                        else:
                            nc.vector.tensor_copy(out=qt_t[:, nc_, :], in_=psq[:])

                    def emit_qproj(j, xt_t):
                        qt_t = qt_pool.tile([P, KC, MB], BF16, tag="qt", name="qt_t")
                        for nc_ in range(KC):
                            emit_qproj_group(qt_t, xt_t, nc_)
                        return qt_t

                    qt_first = emit_qproj(0, xt_first)

                    # ---- prologue: K^T and V from text (PSUM via psq ring) ----
                    wk_sb = ot_pool.tile([P, KC, C], BF16, tag="oud", name="wk_sb")
                    for kc in range(KC):
                        nc.sync.dma_start(wk_sb[:, kc, :], wk_v[:, kc, :])
                    for nc_ in range(KC):
                        psk_full = ps_q.tile([P, MB], F32, tag="psq", name="psk")
                        psk = psk_full[:, :TP]
                        for kc in range(KC):
                            nc.tensor.matmul(
                                psk[:],
                                wk_sb[:, kc, nc_ * P : (nc_ + 1) * P],
                                yt_sb[:, kc, :],
                                start=(kc == 0),
                                stop=(kc == KC - 1),
                            )
                        nc.vector.tensor_copy(out=kt_sb[:, nc_, :], in_=psk[:, :T])

                    wv_sb = ot_pool.tile([P, KC, C], BF16, tag="oud", name="wv_sb")
                    nc.sync.dma_start(wv_sb[:], wv_v[:])
                    for half in range(2):
                        psv_full = ps_q.tile([P, MB], F32, tag="psq", name="psv")
                        psv = psv_full[:T, :]
                        for kc in range(KC):
                            nc.tensor.matmul(
                                psv[:],
                                yt_sb[:, kc, :T],
                                wv_sb[:, kc, half * MB : (half + 1) * MB],
                                start=(kc == 0),
                                stop=(kc == KC - 1),
                            )
                        nc.vector.tensor_copy(
                            out=v_sb[:, half * 8 : (half + 1) * 8, 0:D],
                            in_=psv[:].rearrange("t (h d) -> t h d", d=D),
                        )
                    nc.vector.memset(v_sb[:, :, D : D + 1], 1.0)
                    # wo is first needed when block 0's O-projection runs
                    # (during block 1): its load is issued from inside block
                    # 0's pair loop so the 2 MB transfer stays out of the
                    # bandwidth-bound startup window

                    ob_cur = [None]

                    def oproj_matmuls(j, ot_t, chunk, cc_hi=KC):
                        mi, nh = divmod(chunk, 2)
                        pst = ps_out.tile([P, MB], F32, tag="psout")
                        for cc in range(cc_hi):
                            nc.tensor.matmul(
                                pst[:],
                                ot_t[:, cc, mi * P : (mi + 1) * P],
                                wo_sb[:, cc, nh * MB : (nh + 1) * MB],
                                start=(cc == 0),
                                stop=(cc == KC - 1),
                            )
                        return pst

                    def oproj_finish(j, ot_t, chunk, pst, split_store=False):
                        # row-slab [128, 1024] stores go out as one DMA on the
                        # Activation HW queue (its ob copy just ran there, so
                        # the trigger issues with no wait and no head-of-line
                        # blocking of attention-critical DMAs).  split_store
                        # issues two half-slab stores so the final output DMA
                        # overlaps the last chunk's matmuls.
                        mi, nh = divmod(chunk, 2)
                        if nh == 0:
                            ob_cur[0] = ob_pool.tile(
                                [P, C], BF16, tag="ob", name="ob"
                            )
                        ob = ob_cur[0]
                        on_dve = (nh == 0) if split_store else (mi % 2 == 0)
                        if on_dve:
                            nc.vector.tensor_copy(
                                out=ob[:, nh * MB : (nh + 1) * MB], in_=pst[:]
                            )
                        else:
                            nc.scalar.copy(ob[:, nh * MB : (nh + 1) * MB], pst[:])
                        rows = slice(j * MB + mi * P, j * MB + (mi + 1) * P)
                        if split_store:
                            # half-slab stores on separate HW queues so the
                            # final two transfers overlap
                            eng = nc.sync if nh == 0 else nc.scalar
                            eng.dma_start(
                                out[rows, nh * MB : (nh + 1) * MB],
                                ob[:, nh * MB : (nh + 1) * MB],
                            )
                        elif nh == 1:
                            nc.scalar.dma_start(out[rows, :], ob[:])

                    def emit_oproj_chunk(j, ot_t, chunk):
                        pst = oproj_matmuls(j, ot_t, chunk)
                        oproj_finish(j, ot_t, chunk, pst)

                    def emit_oproj_tail(j, ot_t):
                        # epilogue: six chunks accumulate cc=0..5 first —
                        # borrowing the now-dead psq/pss/pso PSUM slots (same
                        # tags, compatible sizes) for ~7.7us of queued PE work
                        # — so the last two pairs' normalize chains complete
                        # before any chunk needs their cc=6/7 accumulations.
                        HOLD = 6
                        srcs = [
                            (ps_out, "psout"),
                            (ps_out, "psout"),
                            (ps_q, "psq"),
                            (ps_q, "psq"),
                            (ps_o, "pso"),
                            (ps_s, "pss"),
                        ]
                        held = []
                        for c in range(HOLD):
                            pool_c, tag_c = srcs[c]
                            pst = pool_c.tile(
                                [P, MB], F32, tag=tag_c, name="pst_tail"
                            )
                            mi, nh = divmod(c, 2)
                            for cc in range(KC - 2):
                                nc.tensor.matmul(
                                    pst[:],
                                    ot_t[:, cc, mi * P : (mi + 1) * P],
                                    wo_sb[:, cc, nh * MB : (nh + 1) * MB],
                                    start=(cc == 0),
                                    stop=False,
                                )
                            held.append(pst)
                        for c, pst in enumerate(held):
                            mi, nh = divmod(c, 2)
                            for cc in (KC - 2, KC - 1):
                                nc.tensor.matmul(
                                    pst[:],
                                    ot_t[:, cc, mi * P : (mi + 1) * P],
                                    wo_sb[:, cc, nh * MB : (nh + 1) * MB],
                                    start=False,
                                    stop=(cc == KC - 1),
                                )
                            oproj_finish(j, ot_t, c, pst)
                        for c in range(HOLD, 2 * MSUB):
                            pst = oproj_matmuls(j, ot_t, c)
                            oproj_finish(
                                j, ot_t, c, pst, split_store=(c >= 2 * MSUB - 2)
                            )

                    for ji, j in enumerate(range(NBLK)):
                        # prefetch next block's input; for block 0 the issue
                        # is deferred into the pair loop (with wo) to keep
                        # the startup window's HBM bandwidth for the weights
                        # the prologue is actually waiting on
                        xt_next = None
                        if j + 1 < NBLK:
                            xt_next = xt_pool.tile(
                                [P, KC, MB], BF16, tag="xt", name="xt_t"
                            )
                            if ji != 0:
                                nc.sync.dma_start(
                                    xt_next[:, : KC // 2, :],
                                    xt_v[:, : KC // 2, (j + 1) * MB : (j + 2) * MB],
                                )
                                nc.scalar.dma_start(
                                    xt_next[:, KC // 2 :, :],
                                    xt_v[:, KC // 2 :, (j + 1) * MB : (j + 2) * MB],
                                )
                        qt_t = qt_first if ji == 0 else qt_next  # noqa: F821
                        qt_next = None

                        # normalized O^T for this block, written per-pair
                        ot_t = ot_pool.tile(
                            [P, KC, MB], BF16, tag="ot2", bufs=3, name="ot_t"
                        )

                        for jc in range(KC):
                            pss2 = ps_s.tile([T, 2, MB], F32, tag="pss")
                            for hf in range(2):
                                nc.tensor.matmul(
                                    pss2[:, hf, :],
                                    kt_sb[64 * hf : 64 * hf + 64, jc, :],
                                    qt_t[64 * hf : 64 * hf + 64, jc, :],
                                    start=True,
                                    stop=True,
                                )
                            es2 = es_pool.tile([T, 2, MB], BF16, tag="es")
                            nc.scalar.activation(
                                es2[:], pss2[:], AF.Exp, scale=SCALE
                            )
                            pso2 = ps_o.tile([D + 1, 2, MB], F32, tag="pso")
                            for hf in range(2):
                                nc.tensor.matmul(
                                    pso2[:, hf, :],
                                    v_sb[:, 2 * jc + hf, :],
                                    es2[:, hf, :],
                                    start=True,
                                    stop=True,
                                )
                            # pair's unnormalized head outputs + denom rows
                            oud_j = ou_pool.tile(
                                [D + 1, 2, MB], BF16, tag="ou", name="oud_j"
                            )
                            tail_pair = (
                                rep == repeat - 1
                                and j == NBLK - 1
                                and jc == KC - 1
                            )
                            if tail_pair:
                                # final pair: copy halves on ACT and DVE in
                                # parallel so the epilogue chain is short
                                nc.scalar.copy(oud_j[:, 0, :], pso2[:, 0, :])
                                nc.vector.tensor_copy(
                                    out=oud_j[:, 1, :], in_=pso2[:, 1, :]
                                )
                            elif jc % 2 == 0:
                                nc.scalar.copy(oud_j[:], pso2[:])
                            else:
                                nc.vector.tensor_copy(out=oud_j[:], in_=pso2[:])

                            # per-pair normalize: gather denom rows, recip,
                            # broadcast, apply.  Chain is hidden under the
                            # PE's per-pair work; the hf=0 multiply runs on
                            # the otherwise-idle GpSimd except for the final
                            # pair, where DVE keeps the epilogue chain short.
                            def emit_norm(jc, oud_j, tail_pair):
                                dn_j = dn_pool.tile(
                                    [2, MB], BF16, tag="dn", name="dn_j"
                                )
                                # even pairs: the gather rides the ACT HW
                                # queue right behind the pair copy that just
                                # ran there (zero wait, no exp blocking);
                                # odd pairs keep the SP queue
                                geng = nc.scalar if jc % 2 == 0 else nc.sync
                                geng.dma_start(
                                    dn_j[:], oud_j[D : D + 1, :, :]
                                )
                                rc_j = rc_pool.tile(
                                    [2, MB], BF16, tag="rc", name="rc_j"
                                )
                                with nc.allow_low_precision(
                                    reason="softmax recip; rel tol 2e-2"
                                ):
                                    nc.vector.reciprocal(rc_j[:], dn_j[:])
                                rb_j = rb_pool.tile(
                                    [D, 2, MB], BF16, tag="rb", name="rb_j"
                                )
                                for hf in range(2):
                                    nc.sync.dma_start(
                                        rb_j[:, hf, :],
                                        rc_j[hf : hf + 1, None, :].to_broadcast(
                                            (1, D, MB)
                                        ),
                                    )
                                eng0 = nc.vector if tail_pair else nc.gpsimd
                                eng0.tensor_tensor(
                                    ot_t[0:64, jc, :],
                                    oud_j[0:D, 0, :],
                                    rb_j[:, 0, :],
                                    mybir.AluOpType.mult,
                                )
                                eng0.tensor_tensor(
                                    ot_t[64:128, jc, :],
                                    oud_j[0:D, 1, :],
                                    rb_j[:, 1, :],
                                    mybir.AluOpType.mult,
                                )

                            # the LAST pair's normalize is emitted after the
                            # next block's Q-projection: its recip/multiply
                            # carry DMA waits that would head-of-line block
                            # the psq copies behind them in the DVE queue
                            if jc < KC - 1 or j + 1 >= NBLK:
                                emit_norm(jc, oud_j, tail_pair)
                                norm_pending = None
                            else:
                                norm_pending = (jc, oud_j)

                            # previous block, deferred: one O-projection
                            # chunk per pair
                            if prev is not None:
                                emit_oproj_chunk(prev[0], prev[1], jc)

                            # block 0: late-issued loads that would other-
                            # wise crowd the startup bandwidth window
                            if ji == 0:
                                if jc == 1:
                                    nc.scalar.dma_start(wo_sb[:], wo_v[:])
                                if jc == 3 and xt_next is not None:
                                    nc.sync.dma_start(
                                        xt_next[:],
                                        xt_v[:, :, (j + 1) * MB : (j + 2) * MB],
                                    )

                        if j + 1 < NBLK:
                            qt_next = emit_qproj(j + 1, xt_next)
                            if norm_pending is not None:
                                emit_norm(norm_pending[0], norm_pending[1], False)
                        prev = (j, ot_t)

                # epilogue: final repetition's last block
                emit_oproj_tail(prev[0], prev[1])
    nc.finalize()
    return nc


def _get_nc(repeat: int = 1):
    global _CACHED_NC
    if _CACHED_NC is None:
        _CACHED_NC = {}
    if repeat not in _CACHED_NC:
        _CACHED_NC[repeat] = _build(repeat)
    return _CACHED_NC[repeat]


def _bf16(a):
    import ml_dtypes

    return np.asarray(a, dtype=ml_dtypes.bfloat16)


def kernel(video_features, text_features, Wq, Wk, Wv, Wo, **_unused):
    video_features = np.asarray(video_features, dtype=np.float32)
    text_features = np.asarray(text_features, dtype=np.float32)
    wqt = _bf16(np.asarray(Wq, dtype=np.float32).T)
    wkt = _bf16(np.asarray(Wk, dtype=np.float32).T)
    wvt = _bf16(np.asarray(Wv, dtype=np.float32).T)
    wot = _bf16(np.asarray(Wo, dtype=np.float32).T)

    in_maps = []
    for c in range(8):
        b, half = divmod(c, 2)
        xs = video_features[b, half * M : (half + 1) * M, :]  # [M, C]
        in_maps.append(
            {
                "xt": _bf16(np.ascontiguousarray(xs.T)),
                "yt": _bf16(np.ascontiguousarray(text_features[b].T)),
                "wqt": wqt,
                "wkt": wkt,
                "wvt": wvt,
                "wot": wot,
            }
        )

    res = run_bass_kernel_spmd(_get_nc(), in_maps, core_ids=list(range(8)))
    outf = np.empty((B, T_V, C), dtype=np.float32)
    for c in range(8):
        b, half = divmod(c, 2)
        outf[b, half * M : (half + 1) * M, :] = np.asarray(
            res.results[c]["out"], dtype=np.float32
        )
    return outf
